# revision 2
# baseline (speedup 1.0000x reference)
"""AddContextFrames distributed Trainium2 kernel.

out[0, w*80+f, t] = signal[0, f, t + w - 9]  (zero outside), w in 0..18.

Strategy: shard the time axis across 8 NeuronCores. Each core receives a
zero-padded input shard (80, 4096+18) that already includes the halo, so no
inter-core communication is needed. On-core: one DMA load into SBUF, then 19
shifted-window DMA stores into the (1520, 4096) output shard.
"""

import numpy as np

import concourse.bass as bass
import concourse.mybir as mybir
from concourse.bass_utils import run_bass_kernel_spmd

N_CORES = 8
N_CONTEXT = 9
WINDOW = 2 * N_CONTEXT + 1  # 19
FEATS = 80
STEPS = 32768
SHARD = STEPS // N_CORES    # 4096
HALO = 2 * N_CONTEXT        # 18
IN_W = SHARD + HALO         # 4114
OUT_CH = WINDOW * FEATS     # 1520

_nc_cache = None


def build_nc() -> bass.Bass:
    nc = bass.Bass()
    x = nc.declare_dram_parameter(
        "signal", [FEATS, IN_W], mybir.dt.float32, isOutput=False
    )
    out = nc.declare_dram_parameter(
        "out", [OUT_CH, SHARD], mybir.dt.float32, isOutput=True
    )
    with (
        nc.sbuf_tensor([FEATS, IN_W], mybir.dt.float32) as tile,
        nc.semaphore("dma_sem") as dma_sem,
        nc.Block() as block,
    ):
        @block.sync
        def _(sync):
            sync.dma_start(out=tile[:, :], in_=x[:, :]).then_inc(dma_sem, 16)
            sync.wait_ge(dma_sem, 16)
            for w in range(WINDOW):
                sync.dma_start(
                    out=out[w * FEATS : (w + 1) * FEATS, :],
                    in_=tile[:, w : w + SHARD],
                ).then_inc(dma_sem, 16)
            sync.wait_ge(dma_sem, 16 * (1 + WINDOW))

    return nc


def _install_ntff_hook():
    """The image lacks antenv.axon_hooks; synthesize it so trace=True works."""
    import sys, types

    if "antenv.axon_hooks" in sys.modules:
        return
    try:
        from trn_agent_boot.trn_boot import _ntff_profile_via_ctypes

        mod = types.ModuleType("antenv.axon_hooks")
        _state = {"hook": _ntff_profile_via_ctypes("/opt/axon/libaxon_pjrt.so")}
        mod.get_axon_ntff_profile_hook = lambda: _state["hook"]
        mod.set_axon_ntff_profile_hook = lambda h: _state.__setitem__("hook", h)
        sys.modules["antenv.axon_hooks"] = mod
        import antenv

        antenv.axon_hooks = mod
    except Exception:
        pass


def run(signal: np.ndarray, trace: bool = False):
    """signal: (1, 80, 32768) f32 -> ((1, 1520, 32768) f32, exec_time_ns|None)"""
    global _nc_cache
    if trace:
        _install_ntff_hook()
    signal = np.asarray(signal, dtype=np.float32)
    xp = np.zeros((FEATS, STEPS + HALO), np.float32)
    xp[:, N_CONTEXT : N_CONTEXT + STEPS] = signal[0]
    in_maps = [
        {"signal": np.ascontiguousarray(xp[:, i * SHARD : i * SHARD + IN_W])}
        for i in range(N_CORES)
    ]
    if _nc_cache is None:
        _nc_cache = build_nc()
    res = run_bass_kernel_spmd(
        _nc_cache, in_maps, core_ids=list(range(N_CORES)), trace=trace
    )
    out = np.empty((1, OUT_CH, STEPS), np.float32)
    for i in range(N_CORES):
        out[0, :, i * SHARD : (i + 1) * SHARD] = np.asarray(res.results[i]["out"])
    return out, res


def kernel(signal: np.ndarray) -> np.ndarray:
    out, _ = run(signal, trace=False)
    return out


# revision 4
# speedup vs baseline: 1.3964x; 1.3964x over previous
"""AddContextFrames distributed Trainium2 kernel.

out[0, w*80+f, t] = signal[0, f, t + w - 9]  (zero outside), w in 0..18.

Strategy: shard the time axis across 8 NeuronCores. Each core receives a
zero-padded input shard (80, 4096+18) that already includes the halo, so no
inter-core communication is needed. On-core: one DMA load into SBUF, then 19
shifted-window DMA stores into the (1520, 4096) output shard.
"""

import numpy as np

import concourse.bass as bass
import concourse.mybir as mybir
from concourse.bass_utils import run_bass_kernel_spmd

N_CORES = 8
N_CONTEXT = 9
WINDOW = 2 * N_CONTEXT + 1  # 19
FEATS = 80
STEPS = 32768
SHARD = STEPS // N_CORES    # 4096
HALO = 2 * N_CONTEXT        # 18
IN_W = SHARD + HALO         # 4114
OUT_CH = WINDOW * FEATS     # 1520

_nc_cache = None


# Port-balanced SBUF layout: sub-row s = f*8 + b (f feature, b 512-step time
# block) lives at partition s % 128, region r = s // 128 (5 regions), holding
# x[f, b*512 : b*512+530].  All 128 partitions (16 SBUF ports) carry equal
# load, unlike the naive 80-partition layout (62.5% of port bandwidth).
NB = 8               # time sub-blocks per feature
TB = SHARD // NB     # 512
SUBW = TB + HALO     # 530
NR = (FEATS * NB) // 128  # 5 regions per partition
PITCH = 536          # sub-row pitch in elements (32B aligned)


def build_nc() -> bass.Bass:
    from concourse.ap import AP

    nc = bass.Bass()
    x = nc.declare_dram_parameter(
        "signal", [FEATS, IN_W], mybir.dt.float32, isOutput=False
    )
    out = nc.declare_dram_parameter(
        "out", [OUT_CH, SHARD], mybir.dt.float32, isOutput=True
    )
    with (
        nc.sbuf_tensor([128, NR, PITCH], mybir.dt.float32) as tile,
        nc.semaphore("dma_sem") as dma_sem,
        nc.Block() as block,
    ):
        @block.sync
        def _(sync):
            for r in range(NR):
                src = AP(x, r * 16 * IN_W, [[IN_W, 16], [TB, 8], [1, SUBW]])
                sync.dma_start(out=tile[:, r, 0:SUBW], in_=src).then_inc(
                    dma_sem, 16
                )
            n = NR
            for r in range(NR):
                sync.wait_ge(dma_sem, 16 * (r + 1))
                for w in range(WINDOW):
                    dst = AP(
                        out,
                        w * FEATS * SHARD + r * 128 * TB,
                        [[TB, 128], [1, TB]],
                    )
                    sync.dma_start(
                        out=dst, in_=tile[:, r, w : w + TB]
                    ).then_inc(dma_sem, 16)
                    n += 1
            sync.wait_ge(dma_sem, 16 * n)

    return nc


def _install_ntff_hook():
    """The image lacks antenv.axon_hooks; synthesize it so trace=True works."""
    import sys, types

    if "antenv.axon_hooks" in sys.modules:
        return
    try:
        from trn_agent_boot.trn_boot import _ntff_profile_via_ctypes

        mod = types.ModuleType("antenv.axon_hooks")
        _state = {"hook": _ntff_profile_via_ctypes("/opt/axon/libaxon_pjrt.so")}
        mod.get_axon_ntff_profile_hook = lambda: _state["hook"]
        mod.set_axon_ntff_profile_hook = lambda h: _state.__setitem__("hook", h)
        sys.modules["antenv.axon_hooks"] = mod
        import antenv

        antenv.axon_hooks = mod
    except Exception:
        pass


def run(signal: np.ndarray, trace: bool = False):
    """signal: (1, 80, 32768) f32 -> ((1, 1520, 32768) f32, exec_time_ns|None)"""
    global _nc_cache
    if trace:
        _install_ntff_hook()
    signal = np.asarray(signal, dtype=np.float32)
    xp = np.zeros((FEATS, STEPS + HALO), np.float32)
    xp[:, N_CONTEXT : N_CONTEXT + STEPS] = signal[0]
    in_maps = [
        {"signal": np.ascontiguousarray(xp[:, i * SHARD : i * SHARD + IN_W])}
        for i in range(N_CORES)
    ]
    if _nc_cache is None:
        _nc_cache = build_nc()
    res = run_bass_kernel_spmd(
        _nc_cache, in_maps, core_ids=list(range(N_CORES)), trace=trace
    )
    out = np.empty((1, OUT_CH, STEPS), np.float32)
    for i in range(N_CORES):
        out[0, :, i * SHARD : (i + 1) * SHARD] = np.asarray(res.results[i]["out"])
    return out, res


def kernel(signal: np.ndarray) -> np.ndarray:
    out, _ = run(signal, trace=False)
    return out
